# revision 8
# baseline (speedup 1.0000x reference)
"""Trainium2 Bass kernel for DualTimeConstantHighPassMixAdaptation.

Math (reference):
    xr = relu(x)
    Mf[t] = (1-mu_f)*Mf[t-1] + mu_f*xr[t],  Mf[0] = xr[0]   (same for Ms)
    M  = ma*Mf + (1-ma)*Ms,   ma = sigmoid(mix_weight_adapt)
    out = xr/(eps+M) + mh*(xr - M),         mh = sigmoid(mix_weight_hp)

Kernel formulation (all scales folded so the tail is cheap):
    Q = mh*ma*mu_f ; R = mh*(1-ma)*mu_s ; E = mh*eps
    xf = relu(Q*x)                               [ACT]
    xs = xf*(R/Q) + E*mu_s                       [ACT]
    Pf = scan(af, xf)   -> mh*ma*Mf              [DVE]
    Ps = scan(as, xs)   -> mh*((1-ma)*Ms + eps)  [DVE]  (eps is the scan fixed point)
    S  = Pf + Ps        -> mh*(M + eps)          [DVE/GPSIMD]
    q  = 1/S  (fast approx)                      [DVE]
    r2 = q*(mh/Q) + (mh/Q)                       [ACT]
    t1 = r2 * xf        -> (1/(M+eps) + mh)*xr   [GPSIMD/DVE]
    out = t1 - S        (drops +mh*eps ~ 5.7e-7) [GPSIMD/DVE]

Sharding: core b <- batch b (64 channel lanes). In-core the 64 lanes are
split into two time halves -> 128 partitions x 32000 samples. Half-1 rows
get their scan initial state from a prepass that re-scans the last
W samples of half-0 (EMA decay bounds the truncation error ~a^W).
"""

import sys

for _p in ("/opt/trn_rl_repo", "/root/.axon_site/_ro/trn_rl_repo"):
    if _p not in sys.path:
        sys.path.insert(0, _p)

from contextlib import ExitStack

import numpy as np

import concourse.bacc as bacc
import concourse.tile as tile
from concourse import mybir
from concourse.bass_utils import run_bass_kernel_spmd

_dt = mybir.dt.float32
_A = mybir.AluOpType

# Problem geometry (hardcoded per spec).
B, C, T = 8, 64, 64000
HALF = T // 2          # 32000
FT = 1600              # main-loop chunk columns
NCHUNK = HALF // FT    # 20
W_SLOW = 16384         # slow-EMA prepass window
W_FAST = 1024          # fast-EMA prepass window
PRE_FT = 1024
NPRE = W_SLOW // PRE_FT
EPS = np.float32(1e-6)

# engine split for the two flexible tensor-tensor ops (columns to DVE)
SPLIT_T1 = 0     # t1 mult: 0 -> all GPSIMD
SPLIT_OUT = 0    # out sub: 0 -> all GPSIMD


def _f32(v) -> np.float32:
    return np.float32(np.asarray(v).reshape(()))


def _build(consts: dict):
    af = float(consts["af"]); as_ = float(consts["as"])
    Q = float(consts["Q"]); R = float(consts["R"])
    E = float(consts["E"]); mu_s = float(consts["mu_s"])
    mh_ma = float(consts["mh_ma"]); mh_1ma = float(consts["mh_1ma"])
    r2_sc = float(consts["r2_sc"])

    nc = bacc.Bacc("TRN2", target_bir_lowering=False, debug=False, num_devices=B)
    x_d = nc.dram_tensor("x", [C, T], _dt, kind="ExternalInput")
    y_d = nc.dram_tensor("y", [C, T], _dt, kind="ExternalOutput")

    # [half, lane, t] view; DMA walks (half, lane) as the 128 partitions.
    xh = x_d.ap().rearrange("l (h t) -> h l t", h=2)
    yh = y_d.ap().rearrange("l (h t) -> h l t", h=2)

    with tile.TileContext(nc) as tc, ExitStack() as ctx:
        cpool = ctx.enter_context(tc.tile_pool(name="consts", bufs=1))
        af_t = cpool.tile([128, FT], _dt, tag="af")
        as_t = cpool.tile([128, FT], _dt, tag="as")
        nc.vector.memset(af_t[:], af)
        nc.vector.memset(as_t[:], as_)
        as_pre = cpool.tile([128, PRE_FT], _dt, tag="asp")
        nc.vector.memset(as_pre[:], as_)
        init_t = cpool.tile([128, 2], _dt, tag="init")
        init_f = init_t[:, 0:1]
        init_s = init_t[:, 1:2]

        # ---- prepass: recover half-1 scan initials from half-0 tail ----
        ppool = ctx.enter_context(tc.tile_pool(name="pre", bufs=2))
        spool = ctx.enter_context(tc.tile_pool(name="prescan", bufs=2))
        pre_prev = None
        pre_f_end = None
        for k in range(NPRE):
            lo = HALF - W_SLOW + k * PRE_FT
            xp = ppool.tile([128, PRE_FT], _dt, tag="prex")
            nc.sync.dma_start(xp[64:128, :], x_d.ap()[:, lo:lo + PRE_FT])
            if k == NPRE - 1:
                # fast prepass input from this chunk's tail (before in-place relu)
                fin = ppool.tile([128, W_FAST], _dt, tag="fin")
                nc.scalar.activation(
                    fin[64:128, :], xp[64:128, PRE_FT - W_FAST:],
                    mybir.ActivationFunctionType.Relu, scale=Q)
            # slow prep in place: xp <- relu(R*xp)
            nc.scalar.activation(
                xp[64:128, :], xp[64:128, :],
                mybir.ActivationFunctionType.Relu, scale=R)
            po = spool.tile([128, PRE_FT], _dt, tag="preo")
            ini = 0.0 if pre_prev is None else pre_prev[64:128, PRE_FT - 1:PRE_FT]
            nc.vector.tensor_tensor_scan(
                po[64:128, :], as_pre[64:128, :], xp[64:128, :], ini,
                _A.mult, _A.add)
            pre_prev = po
            if k == NPRE - 1:
                fo = spool.tile([128, W_FAST], _dt, tag="fo")
                nc.vector.tensor_tensor_scan(
                    fo[64:128, :], af_t[64:128, :W_FAST], fin[64:128, :], 0.0,
                    _A.mult, _A.add)
                pre_f_end = fo

        # assemble initial-state tiles
        nc.scalar.copy(init_f[64:128, :], pre_f_end[64:128, W_FAST - 1:W_FAST])
        nc.scalar.copy(init_s[64:128, :], pre_prev[64:128, PRE_FT - 1:PRE_FT])

        # ---- main streaming loop ----
        mpool = ctx.enter_context(tc.tile_pool(name="main", bufs=2))
        prev_f = None
        prev_s = None
        for j in range(NCHUNK):
            sl = slice(j * FT, (j + 1) * FT)
            xt = mpool.tile([128, FT], _dt, tag="x")
            nc.sync.dma_start(xt[:], xh[:, :, sl])

            if j == 0:
                # half-0 initials from the very first sample of each lane
                nc.scalar.activation(
                    init_f[0:64, :], xt[0:64, 0:1],
                    mybir.ActivationFunctionType.Relu, scale=mh_ma)
                nc.scalar.activation(
                    init_s[0:64, :], xt[0:64, 0:1],
                    mybir.ActivationFunctionType.Relu, scale=mh_1ma)
                nc.vector.tensor_scalar_add(init_s[:], init_s[:], E)

            xf = mpool.tile([128, FT], _dt, tag="xf")
            nc.scalar.activation(xf[:], xt[:],
                                 mybir.ActivationFunctionType.Relu, scale=Q)
            # xs reuses the x tile (x is dead after the relu above)
            xs = xt
            nc.scalar.activation(xs[:], xf[:],
                                 mybir.ActivationFunctionType.Copy,
                                 scale=R / Q, bias=E * mu_s)

            pf = mpool.tile([128, FT], _dt, tag="pf")
            ini_f = init_f[:, 0:1] if j == 0 else prev_f[:, FT - 1:FT]
            nc.vector.tensor_tensor_scan(pf[:], af_t[:], xf[:], ini_f,
                                         _A.mult, _A.add)
            ps = mpool.tile([128, FT], _dt, tag="ps")
            ini_s = init_s[:, 0:1] if j == 0 else prev_s[:, FT - 1:FT]
            nc.vector.tensor_tensor_scan(ps[:], as_t[:], xs[:], ini_s,
                                         _A.mult, _A.add)
            prev_f, prev_s = pf, ps

            s_t = mpool.tile([128, FT], _dt, tag="s")
            nc.vector.tensor_add(s_t[:], pf[:], ps[:])

            q_t = mpool.tile([128, FT], _dt, tag="q")
            nc.vector.reciprocal_approx_fast(q_t[:], s_t[:])

            r2 = mpool.tile([128, FT], _dt, tag="r2")
            nc.scalar.activation(r2[:], q_t[:],
                                 mybir.ActivationFunctionType.Copy,
                                 scale=r2_sc, bias=r2_sc)

            t1 = q_t  # reuse (q is dead after r2)
            if 0 < SPLIT_T1 < FT:
                nc.vector.tensor_mul(t1[:, :SPLIT_T1], r2[:, :SPLIT_T1],
                                     xf[:, :SPLIT_T1])
                nc.gpsimd.tensor_mul(t1[:, SPLIT_T1:], r2[:, SPLIT_T1:],
                                     xf[:, SPLIT_T1:])
            elif SPLIT_T1 >= FT:
                nc.vector.tensor_mul(t1[:], r2[:], xf[:])
            else:
                nc.gpsimd.tensor_mul(t1[:], r2[:], xf[:])

            o_t = r2  # reuse (r2 is dead after t1)
            if 0 < SPLIT_OUT < FT:
                nc.vector.tensor_sub(o_t[:, :SPLIT_OUT], t1[:, :SPLIT_OUT],
                                     s_t[:, :SPLIT_OUT])
                nc.gpsimd.tensor_sub(o_t[:, SPLIT_OUT:], t1[:, SPLIT_OUT:],
                                     s_t[:, SPLIT_OUT:])
            elif SPLIT_OUT >= FT:
                nc.vector.tensor_sub(o_t[:], t1[:], s_t[:])
            else:
                nc.gpsimd.tensor_sub(o_t[:], t1[:], s_t[:])

            nc.sync.dma_start(yh[:, :, sl], o_t[:])

    nc.compile()
    return nc


_CACHE: dict = {}


def _get_nc(consts: dict):
    key = tuple(sorted(consts.items()))
    if key not in _CACHE:
        _CACHE[key] = _build(consts)
    return _CACHE[key]


def _consts_from_inputs(mu_fast, mu_slow, mix_weight_adapt, mix_weight_hp) -> dict:
    mu_f = _f32(mu_fast)
    mu_s = _f32(mu_slow)
    one = np.float32(1.0)
    ma = np.float32(one / (one + np.exp(np.float32(-mix_weight_adapt))))
    mh = np.float32(one / (one + np.exp(np.float32(-mix_weight_hp))))
    af = one - mu_f
    as_ = one - mu_s
    Q = np.float32(mh * ma * mu_f)
    R = np.float32(mh * (one - ma) * mu_s)
    E = np.float32(mh * EPS)
    d = dict(
        af=float(af),
        Q=float(Q), R=float(R), E=float(E), mu_s=float(mu_s),
        mh_ma=float(np.float32(mh * ma)), mh_1ma=float(np.float32(mh * (one - ma))),
        r2_sc=float(np.float32(mh / Q)),
    )
    d["as"] = float(as_)
    return d


def kernel(x, mu_fast, mu_slow, mix_weight_adapt, mix_weight_hp):
    x = np.asarray(x, dtype=np.float32)
    assert x.shape == (B, C, T), x.shape
    consts = _consts_from_inputs(mu_fast, mu_slow, mix_weight_adapt, mix_weight_hp)
    nc = _get_nc(consts)
    in_maps = [{"x": np.ascontiguousarray(x[b])} for b in range(B)]
    res = run_bass_kernel_spmd(nc, in_maps, core_ids=list(range(B)))
    return np.stack([res.results[b]["y"] for b in range(B)], axis=0)


if __name__ == "__main__":
    rng = np.random.default_rng(0)
    import math
    FS = 16000.0
    x = rng.standard_normal((B, C, T), dtype=np.float32)
    out = kernel(
        x,
        np.float32(1.0 - math.exp(-1.0 / (FS * 2.0 / 1000.0))),
        np.float32(1.0 - math.exp(-1.0 / (FS * 60.0 / 1000.0))),
        np.float32(0.5),
        np.float32(0.3),
    )
    print(out.shape, out.dtype, np.isfinite(out).all())
